# revision 8
# baseline (speedup 1.0000x reference)
"""Compositional attention Trainium2 Bass kernel.

Sharding: 8 cores = 2 batches x 4 search-pairs.
Core c handles batch b=c//4 and searches (2*(c%4), 2*(c%4)+1).
Each core computes a partial output x @ ... @ Wout[slice] for its 128
columns of the S*D=512 concat dim; host sums the 4 partials per batch.

On-chip layout notes (per core):
  qT/kT/rqT: [128, N] with search0 on partitions 0:64, search1 on 64:128.
  Scores are computed transposed (keys on partitions) so exp'd scores can
  stream directly into the PE for the attn@v contraction over keys.
  Softmax is unnormalized: denominators (row sums of exp) come from a
  ones-matmul; normalization is folded into the final R-softmax and
  combine stage (exact algebra, not an approximation).
  R=2 softmax over retrievals is computed as sigmoid of the logit diff.
"""

import sys

for _p in ("/opt/trn_rl_repo",):
    if _p not in sys.path:
        sys.path.insert(0, _p)

from contextlib import ExitStack

import numpy as np

import concourse.bass as bass
import concourse.tile as tile
from concourse import bacc
from concourse import mybir
from concourse.bass import ts
from concourse.bass_utils import run_bass_kernel_spmd
from concourse.masks import make_identity

B, N, DIM, S, R, D = 2, 2048, 1024, 8, 2, 64
NCORES = 8
SPC = 2          # searches per core
SD = SPC * D     # 128 (per-core slice of S*D)
RD = R * D       # 128
P = 128
IBL = 512        # i-block (query block)
NIB = N // IBL   # 4
KC = DIM // P    # 8
NJT = N // P     # 16 key tiles
F32 = mybir.dt.float32
BF16 = mybir.dt.bfloat16
SCALE = float(D) ** -0.5
AF = mybir.ActivationFunctionType
ALU = mybir.AluOpType


def _emit(ctx: ExitStack, tc: tile.TileContext, io):
    nc = tc.nc
    xT, wq, wk, wr, wv, wrk, wout, outp = io

    singles = ctx.enter_context(tc.tile_pool(name="singles", bufs=1))
    ident = singles.tile([P, P], F32)
    make_identity(nc, ident)
    ones_b = singles.tile([P, 1], BF16)
    nc.vector.memset(ones_b, 1.0)
    ones_f = singles.tile([P, 1], F32)
    nc.vector.memset(ones_f, 1.0)

    wq_sb = singles.tile([P, KC, SD], F32)
    wk_sb = singles.tile([P, KC, SD], F32)
    wr_sb = singles.tile([P, KC, SD], F32)
    wv_sb = singles.tile([P, KC, RD], F32)
    for dst, src in ((wq_sb, wq), (wk_sb, wk), (wr_sb, wr), (wv_sb, wv)):
        nc.sync.dma_start(out=dst, in_=src.rearrange("(kc p) m -> p kc m", p=P))
    wrk_sb = singles.tile([D, D], F32)
    nc.sync.dma_start(out=wrk_sb, in_=wrk)
    wout_sb = singles.tile([P, DIM], F32)
    nc.sync.dma_start(out=wout_sb, in_=wout)

    acts = ctx.enter_context(tc.tile_pool(name="acts", bufs=1))
    qT = acts.tile([P, N], F32)
    kT = acts.tile([P, N], F32)
    rqT = acts.tile([P, N], F32)
    vT = acts.tile([P, N], F32)
    vnat = acts.tile([P, NJT, RD], BF16)   # [key-part, key-tile, r*d]
    ret0 = acts.tile([P, N], F32)          # search0 retrievedT (unnormalized)
    ret1 = acts.tile([P, N], F32)          # search1
    rq_lo = acts.tile([64, N], F32)        # search1 rq realigned to parts 0:64
    comp = acts.tile([P, N], F32)          # composed output, stacked searches

    # ---------------- projections ----------------
    with tc.tile_pool(name="xpool", bufs=1) as xpool, \
         tc.tile_pool(name="ppsum", bufs=6, space="PSUM") as ppsum, \
         tc.tile_pool(name="tpsum", bufs=2, space="PSUM") as tpsum:
        xs = xpool.tile([P, KC, N], F32)
        nc.sync.dma_start(out=xs, in_=xT.rearrange("(kc p) n -> p kc n", p=P))
        for wsb, dest in ((wq_sb, qT), (wk_sb, kT), (wr_sb, rqT), (wv_sb, vT)):
            pss = [ppsum.tile([P, IBL], F32, tag="pj", name=f"pj{ib}") for ib in range(NIB)]
            for k in range(KC):
                for ib in range(NIB):
                    nc.tensor.matmul(
                        pss[ib],
                        lhsT=wsb[:, k, :],
                        rhs=xs[:, k, ts(ib, IBL)],
                        start=(k == 0),
                        stop=(k == KC - 1),
                    )
            for ib in range(NIB):
                nc.any.tensor_copy(out=dest[:, ts(ib, IBL)], in_=pss[ib])
        # v to natural [keys, r*d] layout via PE transpose
        for jt in range(NJT):
            tp = tpsum.tile([P, P], F32, tag="tp")
            nc.tensor.transpose(tp, vT[:, ts(jt, P)], ident)
            nc.any.tensor_copy(out=vnat[:, jt, :], in_=tp)
        nc.gpsimd.dma_start(out=rq_lo, in_=rqT[64:128, :])

    # DRAM bounce buffers for per-query scalars ([1,N] <-> [128,N/128] dances)
    dramp = ctx.enter_context(tc.tile_pool(name="dramp", bufs=1, space="DRAM"))
    diff_dr = [dramp.tile([N], F32, tag=f"diff{si}", name=f"diff{si}") for si in range(SPC)]
    sums_dr = [dramp.tile([N], F32, tag=f"sums{si}", name=f"sums{si}") for si in range(SPC)]
    ra0_dr = [dramp.tile([N], F32, tag=f"ra0{si}", name=f"ra0d{si}") for si in range(SPC)]
    ra1_dr = [dramp.tile([N], F32, tag=f"ra1{si}", name=f"ra1d{si}") for si in range(SPC)]

    rets = (ret0, ret1)

    # ---------------- attention ----------------
    expp = ctx.enter_context(tc.tile_pool(name="expp", bufs=2))
    with tc.tile_pool(name="sps0", bufs=1, space="PSUM") as sps0, \
         tc.tile_pool(name="sps1", bufs=1, space="PSUM") as sps1, \
         tc.tile_pool(name="mps", bufs=4, space="PSUM") as mps:
        for ib in range(NIB):
            ets = [expp.tile([P, NJT, IBL], BF16, tag="exp", name=f"exp{si}") for si in range(SPC)]
            spools = (sps0, sps1)
            for jg in range(NJT // 2):
                sp = [spools[si].tile([P, 2, IBL], F32, tag=f"sc{si}", name=f"sc{si}")
                      for si in range(SPC)]
                for h in range(2):
                    jt = 2 * jg + h
                    for si in range(SPC):
                        lo = 64 * si
                        nc.tensor.matmul(
                            sp[si][:, h, :],
                            lhsT=kT[lo:lo + 64, ts(jt, P)],
                            rhs=qT[lo:lo + 64, ts(ib, IBL)],
                            start=True, stop=True,
                        )
                for si in range(SPC):
                    nc.scalar.activation(
                        out=ets[si][:, ts(jg, 2), :], in_=sp[si],
                        func=AF.Exp, scale=SCALE,
                    )
            rt = [mps.tile([P, IBL], F32, tag="mm", name=f"rt{si}") for si in range(SPC)]
            sm = [mps.tile([P, IBL], F32, tag="mm", name=f"sm{si}") for si in range(SPC)]
            for jt in range(NJT):
                for si in range(SPC):
                    nc.tensor.matmul(
                        rt[si], lhsT=vnat[:, jt, :], rhs=ets[si][:, jt, :],
                        start=(jt == 0), stop=(jt == NJT - 1),
                    )
            for jt in range(NJT):
                for si in range(SPC):
                    nc.tensor.matmul(
                        sm[si][:1, :], lhsT=ones_b, rhs=ets[si][:, jt, :],
                        start=(jt == 0), stop=(jt == NJT - 1),
                    )
            for si in range(SPC):
                nc.any.tensor_copy(out=rets[si][:, ts(ib, IBL)], in_=rt[si])
                smst = expp.tile([1, IBL], F32, tag=f"smst{si}", name=f"smst{si}")
                nc.vector.tensor_copy(out=smst, in_=sm[si][:1, :])
                nc.sync.dma_start(out=sums_dr[si][None, ts(ib, IBL)], in_=smst[0:1, :])

    # ---------------- epilogue ----------------
    eps = ctx.enter_context(tc.tile_pool(name="eps", bufs=4, space="PSUM"))
    etmp = ctx.enter_context(tc.tile_pool(name="etmp", bufs=2))
    btmp = ctx.enter_context(tc.tile_pool(name="btmp", bufs=1))
    for si in range(SPC):
        retX = rets[si]
        r1lo = btmp.tile([64, N], F32, tag="r1lo")
        nc.gpsimd.dma_start(out=r1lo, in_=retX[64:128, :])
        rqs = rqT[0:64, :] if si == 0 else rq_lo

        dprod = btmp.tile([64, N], F32, tag="dprod")
        for ib in range(NIB):
            pk0 = eps.tile([P, IBL], F32, tag="ep")
            pk1 = eps.tile([P, IBL], F32, tag="ep")
            nc.tensor.matmul(pk0[:64, :], lhsT=wrk_sb,
                             rhs=retX[0:64, ts(ib, IBL)], start=True, stop=True)
            nc.tensor.matmul(pk1[:64, :], lhsT=wrk_sb,
                             rhs=r1lo[:, ts(ib, IBL)], start=True, stop=True)
            rk0s = etmp.tile([64, IBL], F32, tag="rk0")
            nc.vector.tensor_copy(out=rk0s, in_=pk0[:64, :])
            dsub = etmp.tile([64, IBL], F32, tag="dsub")
            nc.vector.tensor_tensor(dsub, rk0s, pk1[:64, :], ALU.subtract)
            nc.vector.tensor_tensor(dprod[:, ts(ib, IBL)],
                                    rqs[:, ts(ib, IBL)], dsub, ALU.mult)
        for ib in range(NIB):
            pd = eps.tile([P, IBL], F32, tag="ep")
            nc.tensor.matmul(pd[:1, :], lhsT=ones_f[0:64, :],
                             rhs=dprod[:, ts(ib, IBL)], start=True, stop=True)
            pdst = etmp.tile([1, IBL], F32, tag="pdst")
            nc.vector.tensor_copy(out=pdst, in_=pd[:1, :])
            nc.sync.dma_start(out=diff_dr[si][None, ts(ib, IBL)], in_=pdst[0:1, :])

        d128 = etmp.tile([P, N // P], F32, tag="d128")
        s128 = etmp.tile([P, N // P], F32, tag="s128")
        nc.gpsimd.dma_start(out=d128, in_=diff_dr[si].rearrange("(p f) -> p f", p=P))
        nc.gpsimd.dma_start(out=s128, in_=sums_dr[si].rearrange("(p f) -> p f", p=P))
        inv = etmp.tile([P, N // P], F32, tag="inv")
        nc.vector.reciprocal(inv, s128)
        t16 = etmp.tile([P, N // P], F32, tag="t16")
        nc.vector.tensor_tensor(t16, d128, inv, ALU.mult)
        ra0 = etmp.tile([P, N // P], F32, tag="ra0")
        nc.scalar.activation(out=ra0, in_=t16, func=AF.Sigmoid, scale=SCALE)
        ra0s = etmp.tile([P, N // P], F32, tag="ra0s")
        nc.vector.tensor_tensor(ra0s, ra0, inv, ALU.mult)
        ra1s = etmp.tile([P, N // P], F32, tag="ra1s")
        nc.vector.tensor_tensor(ra1s, inv, ra0s, ALU.subtract)
        nc.gpsimd.dma_start(out=ra0_dr[si].rearrange("(p f) -> p f", p=P), in_=ra0s)
        nc.gpsimd.dma_start(out=ra1_dr[si].rearrange("(p f) -> p f", p=P), in_=ra1s)

        bra0 = btmp.tile([64, N], F32, tag="bra0")
        bra1 = btmp.tile([64, N], F32, tag="bra1")
        nc.gpsimd.dma_start(out=bra0, in_=ra0_dr[si][None, :].to_broadcast([64, N]))
        nc.gpsimd.dma_start(out=bra1, in_=ra1_dr[si][None, :].to_broadcast([64, N]))
        t1 = btmp.tile([64, N], F32, tag="t1")
        t2 = btmp.tile([64, N], F32, tag="t2")
        nc.vector.tensor_tensor(t1, bra0, retX[0:64, :], ALU.mult)
        nc.vector.tensor_tensor(t2, bra1, r1lo, ALU.mult)
        if si == 0:
            nc.vector.tensor_tensor(comp[0:64, :], t1, t2, ALU.add)
        else:
            cs1 = btmp.tile([64, N], F32, tag="cs1")
            nc.vector.tensor_tensor(cs1, t1, t2, ALU.add)
            nc.gpsimd.dma_start(out=comp[64:128, :], in_=cs1)

    # ---------------- output projection ----------------
    for nch in range(N // P):
        for h in range(DIM // IBL):
            pw = eps.tile([P, IBL], F32, tag="ep")
            nc.tensor.matmul(pw, lhsT=comp[:, ts(nch, P)],
                             rhs=wout_sb[:, ts(h, IBL)], start=True, stop=True)
            owst = etmp.tile([P, IBL], F32, tag="owst")
            nc.any.tensor_copy(out=owst, in_=pw)
            nc.sync.dma_start(out=outp[ts(nch, P), ts(h, IBL)], in_=owst)


def build_nc():
    nc = bacc.Bacc()
    xT = nc.declare_dram_parameter("xT", [DIM, N], F32, isOutput=False)
    wq = nc.declare_dram_parameter("wq", [DIM, SD], F32, isOutput=False)
    wk = nc.declare_dram_parameter("wk", [DIM, SD], F32, isOutput=False)
    wr = nc.declare_dram_parameter("wr", [DIM, SD], F32, isOutput=False)
    wv = nc.declare_dram_parameter("wv", [DIM, RD], F32, isOutput=False)
    wrk = nc.declare_dram_parameter("wrk", [D, D], F32, isOutput=False)
    wout = nc.declare_dram_parameter("wout", [SD, DIM], F32, isOutput=False)
    outp = nc.declare_dram_parameter("outp", [N, DIM], F32, isOutput=True)
    io = (xT[:], wq[:], wk[:], wr[:], wv[:], wrk[:], wout[:], outp[:])
    with tile.TileContext(nc) as tc:
        with ExitStack() as ctx:
            _emit(ctx, tc, io)
    nc.compile()
    return nc


_CACHE = {}


def _get_nc():
    if "nc" not in _CACHE:
        _CACHE["nc"] = build_nc()
    return _CACHE["nc"]


def make_in_maps(x, Wsq, Wsk, Wrv, Wrq, Wrk, Wout):
    x = np.asarray(x, np.float32)
    in_maps = []
    for c in range(NCORES):
        b = c // 4
        s0 = 2 * (c % 4)
        sl = slice(s0 * D, (s0 + 2) * D)
        in_maps.append({
            "xT": np.ascontiguousarray(x[b].T).astype(np.float32),
            "wq": np.ascontiguousarray(np.asarray(Wsq, np.float32)[:, sl]),
            "wk": np.ascontiguousarray(np.asarray(Wsk, np.float32)[:, sl]),
            "wr": np.ascontiguousarray(np.asarray(Wrq, np.float32)[:, sl]),
            "wv": np.ascontiguousarray(np.asarray(Wrv, np.float32)),
            "wrk": np.ascontiguousarray(np.asarray(Wrk, np.float32)),
            "wout": np.ascontiguousarray(np.asarray(Wout, np.float32)[sl, :]),
        })
    return in_maps


def combine(results):
    out = np.zeros((B, N, DIM), np.float32)
    for c in range(NCORES):
        out[c // 4] += np.asarray(results[c]["outp"], np.float32)
    return out


def kernel(x, Wsq, Wsk, Wrv, Wrq, Wrk, Wout):
    nc = _get_nc()
    in_maps = make_in_maps(x, Wsq, Wsk, Wrv, Wrq, Wrk, Wout)
    res = run_bass_kernel_spmd(nc, in_maps, list(range(NCORES))).results
    return combine(res)


def _install_ntff_shim():
    """Provide antenv.axon_hooks in images that lack it, driving NTFF
    profiling via ctypes into the injected libaxon_pjrt.so."""
    import types
    import ctypes
    import contextlib

    try:
        from antenv.axon_hooks import get_axon_ntff_profile_hook  # noqa
        return
    except ImportError:
        pass
    so_path = "/opt/axon/libaxon_pjrt.so"
    lib = ctypes.CDLL(so_path)
    if not hasattr(lib, "axon_start_nrt_profile"):
        return
    lib.axon_start_nrt_profile.argtypes = [
        ctypes.POINTER(ctypes.c_int64), ctypes.c_size_t]
    lib.axon_start_nrt_profile.restype = ctypes.c_int64
    lib.axon_stop_nrt_profile.argtypes = [ctypes.c_char_p]
    lib.axon_stop_nrt_profile.restype = ctypes.c_int64

    @contextlib.contextmanager
    def _hook(output_dir, device_ids):
        import jax
        jax.devices()
        if device_ids:
            ids = (ctypes.c_int64 * len(device_ids))(*device_ids)
            rc = lib.axon_start_nrt_profile(ids, len(device_ids))
        else:
            rc = lib.axon_start_nrt_profile(None, 0)
        if rc != 0:
            raise RuntimeError(f"axon_start_nrt_profile rc={rc}")
        try:
            yield
        finally:
            n = lib.axon_stop_nrt_profile(str(output_dir).encode())
            print(f"profile: {n} file(s) written to {output_dir}")

    import antenv
    mod = types.ModuleType("antenv.axon_hooks")
    mod.get_axon_ntff_profile_hook = lambda: _hook
    mod.set_axon_ntff_profile_hook = lambda h: None
    sys.modules["antenv.axon_hooks"] = mod
    antenv.axon_hooks = mod


def run_traced(x, Wsq, Wsk, Wrv, Wrq, Wrk, Wout, **kw):
    _install_ntff_shim()
    nc = _get_nc()
    in_maps = make_in_maps(x, Wsq, Wsk, Wrv, Wrq, Wrk, Wout)
    br = run_bass_kernel_spmd(nc, in_maps, list(range(NCORES)), trace=True, **kw)
    return combine(br.results), br


# revision 12
# speedup vs baseline: 1.2690x; 1.2690x over previous
"""Compositional attention Trainium2 Bass kernel (V2: bf16 matmul path).

Sharding: 8 cores = 2 batches x 4 search-pairs.
Core c handles batch b=c//4 and searches (2*(c%4), 2*(c%4)+1); each core
produces a partial output for its 128 columns of the S*D=512 concat dim
(host sums 4 partials per batch).

V2 notes:
  - All hot matmuls run in bf16 (fp32 matmuls are split into 2 passes by
    the compiler = half throughput) with 1024-wide moving operands.
  - Softmax denominators come from a DVE pairwise add tree over the
    exp'd score tiles + one ones-matmul partition reduce (the V1
    per-key-tile ones-matmuls were ~25% of PE time).
  - Normalization is folded into the R-softmax/combine epilogue (exact).
"""

import sys

for _p in ("/opt/trn_rl_repo",):
    if _p not in sys.path:
        sys.path.insert(0, _p)

from contextlib import ExitStack

import ml_dtypes
import numpy as np

import concourse.bass as bass
import concourse.tile as tile
from concourse import bacc
from concourse import mybir
from concourse.bass import ts
from concourse.bass_utils import run_bass_kernel_spmd
from concourse.masks import make_identity

B, N, DIM, S, R, D = 2, 2048, 1024, 8, 2, 64
NCORES = 8
SPC = 2          # searches per core
SD = SPC * D     # 128 (per-core slice of S*D)
RD = R * D       # 128
P = 128
IBL = 512        # i-block (query block)
NIB = N // IBL   # 4
EBL = 512        # epilogue fp32 matmul free dim
NEB = N // EBL   # 4
KC = DIM // P    # 8
NJT = N // P     # 16 key tiles
F32 = mybir.dt.float32
BF16 = mybir.dt.bfloat16
SCALE = float(D) ** -0.5
AF = mybir.ActivationFunctionType
ALU = mybir.AluOpType


def _emit(ctx: ExitStack, tc: tile.TileContext, io):
    nc = tc.nc
    xT, wq, wk, wr, wv, wrk, wout, outp = io

    singles = ctx.enter_context(tc.tile_pool(name="singles", bufs=1))
    ident = singles.tile([P, P], BF16)
    make_identity(nc, ident)
    ones_b = singles.tile([P, 1], BF16)
    nc.vector.memset(ones_b, 1.0)
    ones_f = singles.tile([P, 1], F32)
    nc.vector.memset(ones_f, 1.0)

    wq_sb = singles.tile([P, KC, SD], BF16)
    wk_sb = singles.tile([P, KC, SD], BF16)
    wr_sb = singles.tile([P, KC, SD], BF16)
    wv_sb = singles.tile([P, KC, RD], BF16)
    for dst, src in ((wq_sb, wq), (wk_sb, wk), (wr_sb, wr), (wv_sb, wv)):
        nc.sync.dma_start(out=dst, in_=src.rearrange("(kc p) m -> p kc m", p=P))
    wrk_sb = singles.tile([D, D], F32)
    nc.sync.dma_start(out=wrk_sb, in_=wrk)
    wout_sb = singles.tile([P, DIM], F32)
    nc.sync.dma_start(out=wout_sb, in_=wout)

    acts = ctx.enter_context(tc.tile_pool(name="acts", bufs=1))
    qT = acts.tile([P, N], BF16)
    kT = acts.tile([P, N], BF16)
    rqT = acts.tile([P, N], F32)
    vT = acts.tile([P, N], BF16)
    vnat = acts.tile([P, NJT, RD], BF16)   # [key-part, key-tile, r*d]
    ret0 = acts.tile([P, N], F32)          # search0 retrievedT (unnormalized)
    ret1 = acts.tile([P, N], F32)          # search1
    rq_lo = acts.tile([64, N], F32)        # search1 rq realigned to parts 0:64
    comp = acts.tile([P, N], F32)          # composed output, stacked searches
    red0 = acts.tile([P, N], BF16)         # per-key-tile exp partial sums
    red1 = acts.tile([P, N], BF16)

    # ---------------- projections ----------------
    with tc.tile_pool(name="xpool", bufs=1) as xpool, \
         tc.tile_pool(name="ppsum", bufs=3, space="PSUM") as ppsum, \
         tc.tile_pool(name="tpsum", bufs=2, space="PSUM") as tpsum:
        xs = xpool.tile([P, KC, N], BF16)
        nc.sync.dma_start(out=xs, in_=xT.rearrange("(kc p) n -> p kc n", p=P))
        for wsb, dest in ((wq_sb, qT), (wk_sb, kT), (wr_sb, rqT), (wv_sb, vT)):
            pss = [ppsum.tile([P, IBL], F32, tag="pj", name=f"pj{ib}")
                   for ib in range(NIB)]
            for k in range(KC):
                for ib in range(NIB):
                    nc.tensor.matmul(
                        pss[ib],
                        lhsT=wsb[:, k, :],
                        rhs=xs[:, k, ts(ib, IBL)],
                        start=(k == 0),
                        stop=(k == KC - 1),
                    )
            for ib in range(NIB):
                nc.any.tensor_copy(out=dest[:, ts(ib, IBL)], in_=pss[ib])
        # v to natural [keys, r*d] layout via PE transpose (bf16, single pass)
        for jt in range(NJT):
            tp = tpsum.tile([P, P], BF16, tag="tp")
            nc.tensor.transpose(tp, vT[:, ts(jt, P)], ident)
            nc.any.tensor_copy(out=vnat[:, jt, :], in_=tp)
        nc.gpsimd.dma_start(out=rq_lo, in_=rqT[64:128, :])

    # DRAM bounce buffers for per-query scalars ([1,N] <-> [128,N/128] dances)
    dramp = ctx.enter_context(tc.tile_pool(name="dramp", bufs=1, space="DRAM"))
    diff_dr = [dramp.tile([N], F32, tag=f"diff{si}", name=f"diff{si}")
               for si in range(SPC)]
    sums_dr = [dramp.tile([N], F32, tag=f"sums{si}", name=f"sums{si}")
               for si in range(SPC)]
    ra0_dr = [dramp.tile([N], F32, tag=f"ra0{si}", name=f"ra0d{si}")
              for si in range(SPC)]
    ra1_dr = [dramp.tile([N], F32, tag=f"ra1{si}", name=f"ra1d{si}")
              for si in range(SPC)]

    rets = (ret0, ret1)
    reds = (red0, red1)

    # ---------------- attention ----------------
    with tc.tile_pool(name="expp", bufs=2) as expp, \
         tc.tile_pool(name="trp0", bufs=8) as trp0, \
         tc.tile_pool(name="trp1", bufs=4) as trp1, \
         tc.tile_pool(name="trp2", bufs=2) as trp2, \
         tc.tile_pool(name="scp", bufs=2, space="PSUM") as scp, \
         tc.tile_pool(name="mps", bufs=2, space="PSUM") as mps:
        for ib in range(NIB):
            ets = [expp.tile([P, NJT, IBL], BF16, tag="exp", name=f"exp{si}")
                   for si in range(SPC)]
            for jg in range(NJT // 2):
                for si in range(SPC):
                    lo = 64 * si
                    sp = scp.tile([P, 2, IBL], F32, tag="sc", name=f"sc{si}")
                    for h in range(2):
                        jt = 2 * jg + h
                        nc.tensor.matmul(
                            sp[:, h, :],
                            lhsT=kT[lo:lo + 64, ts(jt, P)],
                            rhs=qT[lo:lo + 64, ts(ib, IBL)],
                            start=True, stop=True,
                        )
                    nc.scalar.activation(
                        out=ets[si][:, ts(jg, 2), :], in_=sp,
                        func=AF.Exp, scale=SCALE,
                    )
            rt = [mps.tile([P, IBL], F32, tag="mm", name=f"rt{si}")
                  for si in range(SPC)]
            for jt in range(NJT):
                for si in range(SPC):
                    nc.tensor.matmul(
                        rt[si], lhsT=vnat[:, jt, :], rhs=ets[si][:, jt, :],
                        start=(jt == 0), stop=(jt == NJT - 1),
                    )
            for si in range(SPC):
                nc.any.tensor_copy(out=rets[si][:, ts(ib, IBL)], in_=rt[si])
            # DVE pairwise tree over key tiles -> per-query partial sums
            tpools = {8: trp0, 4: trp1, 2: trp2}
            for si in range(SPC):
                lvl = [ets[si][:, jt, :] for jt in range(NJT)]
                while len(lvl) > 2:
                    nxt = []
                    pool = tpools[len(lvl) // 2]
                    for i in range(len(lvl) // 2):
                        t = pool.tile([P, IBL], BF16, tag=f"tr{len(lvl)}",
                                      name=f"tr{si}_{i}")
                        nc.vector.tensor_tensor(t, lvl[2 * i], lvl[2 * i + 1],
                                                ALU.add)
                        nxt.append(t)
                    lvl = nxt
                nc.vector.tensor_tensor(reds[si][:, ts(ib, IBL)],
                                        lvl[0], lvl[1], ALU.add)

    # ---------------- epilogue ----------------
    eps = ctx.enter_context(tc.tile_pool(name="eps", bufs=4, space="PSUM"))
    etmp = ctx.enter_context(tc.tile_pool(name="etmp", bufs=2))
    btmp = ctx.enter_context(tc.tile_pool(name="btmp", bufs=1))
    for si in range(SPC):
        retX = rets[si]
        # partition-reduce of per-key-tile sums -> softmax denominators
        for ib in range(NIB):
            psm = eps.tile([P, IBL], F32, tag="ep", name="psm")
            nc.tensor.matmul(psm[:1, :], lhsT=ones_b,
                             rhs=reds[si][:, ts(ib, IBL)], start=True,
                             stop=True)
            smst = etmp.tile([1, IBL], F32, tag="smst")
            nc.vector.tensor_copy(out=smst, in_=psm[:1, :])
            nc.sync.dma_start(out=sums_dr[si][None, ts(ib, IBL)],
                              in_=smst[0:1, :])

        r1lo = btmp.tile([64, N], F32, tag="r1lo")
        nc.gpsimd.dma_start(out=r1lo, in_=retX[64:128, :])
        rqs = rqT[0:64, :] if si == 0 else rq_lo

        dprod = btmp.tile([64, N], F32, tag="dprod")
        for ib in range(NEB):
            pk0 = eps.tile([P, IBL], F32, tag="ep", name="pk0")
            pk1 = eps.tile([P, IBL], F32, tag="ep", name="pk1")
            nc.tensor.matmul(pk0[:64, :EBL], lhsT=wrk_sb,
                             rhs=retX[0:64, ts(ib, EBL)], start=True, stop=True)
            nc.tensor.matmul(pk1[:64, :EBL], lhsT=wrk_sb,
                             rhs=r1lo[:, ts(ib, EBL)], start=True, stop=True)
            rk0s = etmp.tile([64, EBL], F32, tag="rk0")
            nc.vector.tensor_copy(out=rk0s, in_=pk0[:64, :EBL])
            dsub = etmp.tile([64, EBL], F32, tag="dsub")
            nc.vector.tensor_tensor(dsub, rk0s, pk1[:64, :EBL], ALU.subtract)
            nc.vector.tensor_tensor(dprod[:, ts(ib, EBL)],
                                    rqs[:, ts(ib, EBL)], dsub, ALU.mult)
        for ib in range(NEB):
            pd = eps.tile([P, IBL], F32, tag="ep", name="pd")
            nc.tensor.matmul(pd[:1, :EBL], lhsT=ones_f[0:64, :],
                             rhs=dprod[:, ts(ib, EBL)], start=True, stop=True)
            pdst = etmp.tile([1, EBL], F32, tag="pdst")
            nc.vector.tensor_copy(out=pdst, in_=pd[:1, :EBL])
            nc.sync.dma_start(out=diff_dr[si][None, ts(ib, EBL)],
                              in_=pdst[0:1, :])

        d128 = etmp.tile([P, N // P], F32, tag="d128")
        s128 = etmp.tile([P, N // P], F32, tag="s128")
        nc.gpsimd.dma_start(out=d128,
                            in_=diff_dr[si].rearrange("(p f) -> p f", p=P))
        nc.gpsimd.dma_start(out=s128,
                            in_=sums_dr[si].rearrange("(p f) -> p f", p=P))
        inv = etmp.tile([P, N // P], F32, tag="inv")
        nc.vector.reciprocal(inv, s128)
        t16 = etmp.tile([P, N // P], F32, tag="t16")
        nc.vector.tensor_tensor(t16, d128, inv, ALU.mult)
        ra0 = etmp.tile([P, N // P], F32, tag="ra0")
        nc.scalar.activation(out=ra0, in_=t16, func=AF.Sigmoid, scale=SCALE)
        ra0s = etmp.tile([P, N // P], F32, tag="ra0s")
        nc.vector.tensor_tensor(ra0s, ra0, inv, ALU.mult)
        ra1s = etmp.tile([P, N // P], F32, tag="ra1s")
        nc.vector.tensor_tensor(ra1s, inv, ra0s, ALU.subtract)
        nc.gpsimd.dma_start(out=ra0_dr[si].rearrange("(p f) -> p f", p=P),
                            in_=ra0s)
        nc.gpsimd.dma_start(out=ra1_dr[si].rearrange("(p f) -> p f", p=P),
                            in_=ra1s)

        bra0 = btmp.tile([64, N], F32, tag="bra0")
        bra1 = btmp.tile([64, N], F32, tag="bra1")
        nc.gpsimd.dma_start(out=bra0,
                            in_=ra0_dr[si][None, :].to_broadcast([64, N]))
        nc.gpsimd.dma_start(out=bra1,
                            in_=ra1_dr[si][None, :].to_broadcast([64, N]))
        t1 = btmp.tile([64, N], F32, tag="t1")
        t2 = btmp.tile([64, N], F32, tag="t2")
        nc.vector.tensor_tensor(t1, bra0, retX[0:64, :], ALU.mult)
        nc.vector.tensor_tensor(t2, bra1, r1lo, ALU.mult)
        if si == 0:
            nc.vector.tensor_tensor(comp[0:64, :], t1, t2, ALU.add)
        else:
            cs1 = btmp.tile([64, N], F32, tag="cs1")
            nc.vector.tensor_tensor(cs1, t1, t2, ALU.add)
            nc.gpsimd.dma_start(out=comp[64:128, :], in_=cs1)

    # ---------------- output projection ----------------
    for nch in range(N // P):
        for h in range(DIM // EBL):
            pw = eps.tile([P, IBL], F32, tag="ep", name="pw")
            nc.tensor.matmul(pw[:, :EBL], lhsT=comp[:, ts(nch, P)],
                             rhs=wout_sb[:, ts(h, EBL)], start=True, stop=True)
            owst = etmp.tile([P, EBL], F32, tag="owst")
            nc.any.tensor_copy(out=owst, in_=pw[:, :EBL])
            nc.sync.dma_start(out=outp[ts(nch, P), ts(h, EBL)], in_=owst)


def build_nc():
    nc = bacc.Bacc()
    xT = nc.declare_dram_parameter("xT", [DIM, N], BF16, isOutput=False)
    wq = nc.declare_dram_parameter("wq", [DIM, SD], BF16, isOutput=False)
    wk = nc.declare_dram_parameter("wk", [DIM, SD], BF16, isOutput=False)
    wr = nc.declare_dram_parameter("wr", [DIM, SD], BF16, isOutput=False)
    wv = nc.declare_dram_parameter("wv", [DIM, RD], BF16, isOutput=False)
    wrk = nc.declare_dram_parameter("wrk", [D, D], F32, isOutput=False)
    wout = nc.declare_dram_parameter("wout", [SD, DIM], F32, isOutput=False)
    outp = nc.declare_dram_parameter("outp", [N, DIM], F32, isOutput=True)
    io = (xT[:], wq[:], wk[:], wr[:], wv[:], wrk[:], wout[:], outp[:])
    with tile.TileContext(nc) as tc:
        with ExitStack() as ctx:
            _emit(ctx, tc, io)
    nc.compile()
    return nc


_CACHE = {}


def _get_nc():
    if "nc" not in _CACHE:
        _CACHE["nc"] = build_nc()
    return _CACHE["nc"]


def make_in_maps(x, Wsq, Wsk, Wrv, Wrq, Wrk, Wout):
    x = np.asarray(x, np.float32)
    bf = ml_dtypes.bfloat16
    in_maps = []
    for c in range(NCORES):
        b = c // 4
        s0 = 2 * (c % 4)
        sl = slice(s0 * D, (s0 + 2) * D)
        in_maps.append({
            "xT": np.ascontiguousarray(x[b].T).astype(bf),
            "wq": np.ascontiguousarray(np.asarray(Wsq, np.float32)[:, sl]).astype(bf),
            "wk": np.ascontiguousarray(np.asarray(Wsk, np.float32)[:, sl]).astype(bf),
            "wr": np.ascontiguousarray(np.asarray(Wrq, np.float32)[:, sl]).astype(bf),
            "wv": np.ascontiguousarray(np.asarray(Wrv, np.float32)).astype(bf),
            "wrk": np.ascontiguousarray(np.asarray(Wrk, np.float32)),
            "wout": np.ascontiguousarray(np.asarray(Wout, np.float32)[sl, :]),
        })
    return in_maps


def combine(results):
    out = np.zeros((B, N, DIM), np.float32)
    for c in range(NCORES):
        out[c // 4] += np.asarray(results[c]["outp"], np.float32)
    return out


def kernel(x, Wsq, Wsk, Wrv, Wrq, Wrk, Wout):
    nc = _get_nc()
    in_maps = make_in_maps(x, Wsq, Wsk, Wrv, Wrq, Wrk, Wout)
    res = run_bass_kernel_spmd(nc, in_maps, list(range(NCORES))).results
    return combine(res)


def _install_ntff_shim():
    """Provide antenv.axon_hooks in images that lack it, driving NTFF
    profiling via ctypes into the injected libaxon_pjrt.so."""
    import types
    import ctypes
    import contextlib

    try:
        from antenv.axon_hooks import get_axon_ntff_profile_hook  # noqa
        return
    except ImportError:
        pass
    so_path = "/opt/axon/libaxon_pjrt.so"
    lib = ctypes.CDLL(so_path)
    if not hasattr(lib, "axon_start_nrt_profile"):
        return
    lib.axon_start_nrt_profile.argtypes = [
        ctypes.POINTER(ctypes.c_int64), ctypes.c_size_t]
    lib.axon_start_nrt_profile.restype = ctypes.c_int64
    lib.axon_stop_nrt_profile.argtypes = [ctypes.c_char_p]
    lib.axon_stop_nrt_profile.restype = ctypes.c_int64

    @contextlib.contextmanager
    def _hook(output_dir, device_ids):
        import jax
        jax.devices()
        if device_ids:
            ids = (ctypes.c_int64 * len(device_ids))(*device_ids)
            rc = lib.axon_start_nrt_profile(ids, len(device_ids))
        else:
            rc = lib.axon_start_nrt_profile(None, 0)
        if rc != 0:
            raise RuntimeError(f"axon_start_nrt_profile rc={rc}")
        try:
            yield
        finally:
            n = lib.axon_stop_nrt_profile(str(output_dir).encode())
            print(f"profile: {n} file(s) written to {output_dir}")

    import antenv
    mod = types.ModuleType("antenv.axon_hooks")
    mod.get_axon_ntff_profile_hook = lambda: _hook
    mod.set_axon_ntff_profile_hook = lambda h: None
    sys.modules["antenv.axon_hooks"] = mod
    antenv.axon_hooks = mod


def run_traced(x, Wsq, Wsk, Wrv, Wrq, Wrk, Wout, **kw):
    _install_ntff_shim()
    nc = _get_nc()
    in_maps = make_in_maps(x, Wsq, Wsk, Wrv, Wrq, Wrk, Wout)
    br = run_bass_kernel_spmd(nc, in_maps, list(range(NCORES)), trace=True, **kw)
    return combine(br.results), br
